# revision 1
# baseline (speedup 1.0000x reference)
"""ChamferLoss kernel for 8 Trainium2 NeuronCores.

Problem: pred (4,8192,3) f32, gt (4,8192,3) f32 ->
  loss = mean_b[ mean(pred2gt_b) + mean(gt2pred_b) + max(pred2gt_b) ]   (scalar f32)
where pred2gt[b,i] = min_j ||pred[b,i]-gt[b,j]||^2 and gt2pred[b,j] = min_i (same).

Sharding: data-parallel over B (2 cores per batch) x sequence-sharded rows.
Each core computes row-mins of two 4096x8192 distance blocks (dual orientation:
pred-half vs gt-full, and gt-half vs pred-full), so ALL reductions are free-axis
row reductions; the host does only the tiny final mean/max combines.

Distance computation: one K=18 bf16 matmul per tile via the augmented split-
precision form  d = nx + ny - 2 x.y  with x = xh + xl (bf16 hi/lo split) and
norms split into 3 bf16 parts; PSUM accumulates in fp32, so results are
fp32-accurate (abs err ~1e-4, dominated by the bf16-split representation).

Reduction: per 2048-column PSUM unit, ScalarE copies the upper 1024 columns to
SBUF; a custom fused DVE op (min body + min accumulate) then reduces the lower
1024 PSUM columns against the SBUF copy in a single 1x pass, draining PSUM
through both the DVE and ACT read ports concurrently.
"""

from contextlib import ExitStack

import numpy as np
import ml_dtypes

import concourse.bass as bass
import concourse.tile as tile
from concourse import bacc, mybir
from concourse import dve_ops
from concourse.bass_utils import run_bass_kernel_spmd
from concourse.dve_ops import DveOp
from concourse.dve_spec import Spec, Src0, Src1, C0, minn, lower
from concourse.dve_uop import DveOpSpec

B = 4
N = 8192          # pred points per batch
M = 8192          # gt points per batch
NCORES = 8
HALF = 4096       # rows per core per orientation
K = 18            # augmented contraction rows
ITILE = 128       # rows per matmul tile
NSTRIP = 512      # matmul moving free dim
UNIT = 2048       # columns per psum unit (4 matmuls, 4 banks)
NUNITS = M // UNIT          # 4 units per i-tile
NITILES = HALF // ITILE     # 32 i-tiles per orientation
BIG = 3.0e38

_bf16 = ml_dtypes.bfloat16


# --------------------------------------------------------------------------- #
# Custom fused DVE op: out = min(in0, in1); accum_out = min(s0, min_k out)
# --------------------------------------------------------------------------- #

def _ttmin_ref(in0, in1, s0, s1, imm2):
    out = np.minimum(in0.astype(np.float32), in1.astype(np.float32))
    s0v = s0 if np.ndim(s0) == 0 else np.asarray(s0).reshape(-1)
    return out, np.minimum(out.min(axis=-1), s0v)


def _register_min_op() -> DveOp:
    name = "TT_MIN_RED_ANT"
    for o in dve_ops.OPS:
        if o.name == name:
            return o
    spec = Spec(body=minn(Src0, Src1), accum=minn, accum_init=C0, reference=_ttmin_ref)
    shas = {}
    for ver in ("v3", "v4"):
        try:
            s = DveOpSpec(name=name, opcode=0, uops=lower(spec, ver=ver), rd1_en=True)
            shas[ver] = s.sha(ver)
        except Exception:
            pass
    op = DveOp(name, spec, subdim=False, uops_sha=shas)
    dve_ops.OPS.append(op)
    dve_ops._SUB_OPCODE_FOR_NAME[name] = dve_ops._CUSTOM_DVE_ROW_BASE + len(dve_ops.OPS) - 1
    dve_ops.CUSTOM_DVE_SPECS[name] = spec
    return op


# --------------------------------------------------------------------------- #
# Bass program (identical SPMD program on all 8 cores)
# --------------------------------------------------------------------------- #

_CACHE: dict = {}
VARIANT = "pack2d"


def _build_program(reps: int = 1, loop: int = 1, variant: str = "split"):
    """variant: 'split' (ACT copy + custom DVE), 'dve' (plain DVE reduce only)."""
    op = _register_min_op()
    nc = bacc.Bacc("TRN2", target_bir_lowering=False, debug=False, num_devices=NCORES)

    packed = variant.startswith("pack2")
    KP = 32 + K if packed else K  # packed lhsT/rhs carry rows at partitions 0..17 and 32..49
    LW = HALF if variant.startswith("pack2d") else ((HALF // 2) if packed else HALF)
    ins = {}
    outs = {}
    for o in ("E", "F"):
        ins[f"lhsT_{o}"] = nc.dram_tensor(
            f"lhsT_{o}", [KP, LW], mybir.dt.bfloat16, kind="ExternalInput").ap()
        ins[f"rhs_{o}"] = nc.dram_tensor(
            f"rhs_{o}", [KP, M], mybir.dt.bfloat16, kind="ExternalInput").ap()
        outs[o] = nc.dram_tensor(
            f"out{o}", [ITILE, NITILES], mybir.dt.float32, kind="ExternalOutput").ap()

    with tile.TileContext(nc) as tc:
        with ExitStack() as ctx:
            if loop > 1:
                ctx.enter_context(tc.For_i(0, loop, 1))
            inp = ctx.enter_context(tc.tile_pool(name="inp", bufs=2))
            psum = ctx.enter_context(tc.tile_pool(name="psum", bufs=2, space="PSUM"))
            acp = ctx.enter_context(tc.tile_pool(name="acp", bufs=3))
            scr = ctx.enter_context(tc.tile_pool(name="scr", bufs=3))
            stp = ctx.enter_context(tc.tile_pool(name="stp", bufs=3))
            ost = ctx.enter_context(tc.tile_pool(name="ost", bufs=1))

            for o in ("E", "F") * reps:
                lhsT = inp.tile([KP, LW], mybir.dt.bfloat16, tag="lhsT")
                nc.sync.dma_start(out=lhsT[:], in_=ins[f"lhsT_{o}"][:])
                rhs = inp.tile([KP, M], mybir.dt.bfloat16, tag="rhs")
                nc.sync.dma_start(out=rhs[:], in_=ins[f"rhs_{o}"][:])

                if variant.startswith("pack2d"):
                    outstage = ost.tile([ITILE, NITILES], mybir.dt.float32,
                                        tag="outstage")
                    for t in range(NITILES):
                        strip = stp.tile([ITILE, 4], mybir.dt.float32, tag="strip")
                        cp = None
                        for u in range(8):  # 1024-col units (2 strips, one per group)
                            pt = psum.tile([ITILE, 1024], mybir.dt.float32,
                                           tag="pt", bufs=4)
                            for g in range(2):
                                j0 = (2 * u + g) * NSTRIP
                                nc.tensor.matmul(
                                    pt[:, g * NSTRIP:(g + 1) * NSTRIP],
                                    lhsT[32 * g:32 * g + K,
                                         t * ITILE:(t + 1) * ITILE],
                                    rhs[32 * g:32 * g + K, j0:j0 + NSTRIP],
                                    start=True, stop=True)
                            if u % 2 == 0:
                                cp = acp.tile([ITILE, 1024], mybir.dt.float32,
                                              tag="cp")
                                nc.scalar.copy(cp[:], pt[:])
                            else:
                                sc = scr.tile([ITILE, 1024], mybir.dt.bfloat16,
                                              tag="sc")
                                nc.vector._custom_dve(
                                    op, out=sc[:], in0=pt[:], in1=cp[:],
                                    s0=BIG,
                                    accum_out=strip[:, u // 2:u // 2 + 1])
                        nc.vector.tensor_reduce(
                            outstage[:, t:t + 1], strip[:],
                            axis=mybir.AxisListType.X, op=mybir.AluOpType.min)
                    nc.sync.dma_start(out=outs[o][:], in_=outstage[:])
                    continue

                if packed:
                    outstage = ost.tile([ITILE, NITILES], mybir.dt.float32,
                                        tag="outstage")
                    NS = NITILES // 2  # 16 supertiles of 2 i-tiles
                    for s in range(NS):
                        strip = stp.tile([ITILE, 2, 4], mybir.dt.float32, tag="strip")
                        cp = None
                        for u in range(8):  # units of 2 j-strips
                            pt = psum.tile([ITILE, 2048], mybir.dt.float32, tag="pt")
                            for jj in range(2):
                                for g in range(2):
                                    j0 = (u * 2 + jj) * NSTRIP
                                    nc.tensor.matmul(
                                        pt[:, g * 1024 + jj * NSTRIP:
                                           g * 1024 + (jj + 1) * NSTRIP],
                                        lhsT[32 * g:32 * g + K,
                                             s * ITILE:(s + 1) * ITILE],
                                        rhs[32 * g:32 * g + K, j0:j0 + NSTRIP],
                                        start=True, stop=True)
                            if variant == "pack2_cheap":
                                nc.vector.tensor_reduce(
                                    strip[:, u % 2, u // 2:u // 2 + 1],
                                    pt[:, 0:64],
                                    axis=mybir.AxisListType.X,
                                    op=mybir.AluOpType.min)
                                continue
                            if variant == "pack2_mm":
                                continue
                            if u % 2 == 0:
                                cp = acp.tile([ITILE, 2048], mybir.dt.float32,
                                              tag="cp")
                                nc.scalar.copy(cp[:], pt[:])
                            else:
                                for g in range(2):
                                    sc = scr.tile([ITILE, 1024], mybir.dt.bfloat16,
                                                  tag="sc")
                                    nc.vector._custom_dve(
                                        op, out=sc[:],
                                        in0=pt[:, g * 1024:(g + 1) * 1024],
                                        in1=cp[:, g * 1024:(g + 1) * 1024],
                                        s0=BIG,
                                        accum_out=strip[:, g, u // 2:u // 2 + 1])
                        if variant == "pack2_mm":
                            nc.vector.memset(outstage[:, 2 * s:2 * s + 2], 0.0)
                        else:
                            for g in range(2):
                                nc.vector.tensor_reduce(
                                    outstage[:, 2 * s + g:2 * s + g + 1],
                                    strip[:, g, :],
                                    axis=mybir.AxisListType.X, op=mybir.AluOpType.min)
                    nc.sync.dma_start(out=outs[o][:], in_=outstage[:])
                    continue

                outstage = ost.tile([ITILE, NITILES], mybir.dt.float32, tag="outstage")
                if variant in ("mm", "mm_act"):
                    nc.vector.memset(outstage[:], 0.0)
                if variant == "mm_dvec":
                    cp0 = acp.tile([ITILE, UNIT // 2], mybir.dt.float32, tag="cp0")
                    nc.vector.memset(cp0[:], 0.0)
                for t in range(NITILES):
                    w = lhsT[:, t * ITILE:(t + 1) * ITILE]
                    strip = stp.tile([ITILE, NUNITS], mybir.dt.float32, tag="strip")
                    for u in range(NUNITS):
                        pt = psum.tile([ITILE, UNIT], mybir.dt.float32, tag="pt")
                        for k in range(UNIT // NSTRIP):
                            j0 = u * UNIT + k * NSTRIP
                            nc.tensor.matmul(
                                pt[:, k * NSTRIP:(k + 1) * NSTRIP],
                                w, rhs[:, j0:j0 + NSTRIP],
                                start=True, stop=True)
                        if variant == "split":
                            cp = acp.tile([ITILE, UNIT // 2], mybir.dt.float32, tag="cp")
                            nc.scalar.copy(cp[:], pt[:, UNIT // 2:UNIT])
                            sc = scr.tile([ITILE, UNIT // 2], mybir.dt.bfloat16, tag="sc")
                            nc.vector._custom_dve(
                                op, out=sc[:], in0=pt[:, 0:UNIT // 2], in1=cp[:],
                                s0=BIG, accum_out=strip[:, u:u + 1])
                        elif variant == "dve":
                            nc.vector.tensor_reduce(
                                strip[:, u:u + 1], pt[:],
                                axis=mybir.AxisListType.X, op=mybir.AluOpType.min)
                        elif variant == "cheap":
                            nc.vector.tensor_reduce(
                                strip[:, u:u + 1], pt[:, 0:64],
                                axis=mybir.AxisListType.X, op=mybir.AluOpType.min)
                        elif variant == "mm":
                            pass  # PE only
                        elif variant == "mm_act":
                            cp = acp.tile([ITILE, UNIT // 2], mybir.dt.float32, tag="cp")
                            nc.scalar.copy(cp[:], pt[:, UNIT // 2:UNIT])
                        elif variant == "mm_dvec":
                            sc = scr.tile([ITILE, UNIT // 2], mybir.dt.bfloat16, tag="sc")
                            nc.vector._custom_dve(
                                op, out=sc[:], in0=pt[:, 0:UNIT // 2], in1=cp0[:],
                                s0=BIG, accum_out=strip[:, u:u + 1])
                    if variant in ("split", "dve", "mm_dvec", "cheap"):
                        nc.vector.tensor_reduce(
                            outstage[:, t:t + 1], strip[:],
                            axis=mybir.AxisListType.X, op=mybir.AluOpType.min)
                nc.sync.dma_start(out=outs[o][:], in_=outstage[:])

    nc.compile()
    return nc


# --------------------------------------------------------------------------- #
# Host-side input prep: augmented split-precision matrices
# --------------------------------------------------------------------------- #

def _split3(v):
    """Split fp32/fp64 array into 3 bf16 parts summing to ~v."""
    a = v.astype(_bf16).astype(np.float64)
    r = v - a
    b = r.astype(np.float32).astype(_bf16).astype(np.float64)
    c = (r - b).astype(np.float32).astype(_bf16).astype(np.float64)
    return a, b, c


def _augment(xrows, ycols):
    """Build (lhsT [K, nx], rhs [K, ny]) bf16 so that lhsT.T @ rhs [i,j]
    ~= ||x_i - y_j||^2 in fp32 precision.  xrows (nx,3), ycols (ny,3) f32."""
    nx_, ny_ = xrows.shape[0], ycols.shape[0]
    xh = xrows.astype(_bf16).astype(np.float64)
    xl32 = (xrows.astype(np.float64) - xh).astype(np.float32)
    xl = xl32.astype(_bf16).astype(np.float64)
    yh = ycols.astype(_bf16).astype(np.float64)
    yl32 = (ycols.astype(np.float64) - yh).astype(np.float32)
    yl = yl32.astype(_bf16).astype(np.float64)

    xe = xh + xl          # effective points (exactly representable as bf16+bf16)
    ye = yh + yl
    nxv = (xe * xe).sum(1)
    nyv = (ye * ye).sum(1)
    nxa, nxb, nxc = _split3(nxv)
    nya, nyb, nyc = _split3(nyv)

    lhsT = np.zeros((K, nx_), np.float32)
    rhs = np.zeros((K, ny_), np.float32)
    lhsT[0:3] = xh.T; rhs[0:3] = -2.0 * yh.T
    lhsT[3:6] = xh.T; rhs[3:6] = -2.0 * yl.T
    lhsT[6:9] = xl.T; rhs[6:9] = -2.0 * yh.T
    lhsT[9:12] = xl.T; rhs[9:12] = -2.0 * yl.T
    lhsT[12] = nxa; rhs[12] = 1.0
    lhsT[13] = nxb; rhs[13] = 1.0
    lhsT[14] = nxc; rhs[14] = 1.0
    lhsT[15] = 1.0; rhs[15] = nya
    lhsT[16] = 1.0; rhs[16] = nyb
    lhsT[17] = 1.0; rhs[17] = nyc
    return lhsT.astype(_bf16), rhs.astype(_bf16)


def _pack2d(lhsT, rhs):
    """Duplicate all K rows into PE row groups 0 and 32 (same i-tile both groups)."""
    KP = 32 + K
    lp = np.zeros((KP, HALF), np.float32).astype(_bf16)
    lp[0:K] = lhsT
    lp[32:32 + K] = lhsT
    rp = np.zeros((KP, M), np.float32).astype(_bf16)
    rp[0:K] = rhs
    rp[32:32 + K] = rhs
    return lp, rp


def _pack2(lhsT, rhs):
    """Interleave pairs of i-tiles into PE row groups 0 and 32."""
    KP = 32 + K
    lp = np.zeros((KP, HALF // 2), np.float32).astype(_bf16)
    v = np.asarray(lhsT).reshape(K, NITILES // 2, 2, ITILE)
    lp[0:K] = v[:, :, 0, :].reshape(K, HALF // 2)
    lp[32:32 + K] = v[:, :, 1, :].reshape(K, HALF // 2)
    rp = np.zeros((KP, M), np.float32).astype(_bf16)
    rp[0:K] = rhs
    rp[32:32 + K] = rhs
    return lp, rp


def _make_in_maps(pred, gt, variant="split"):
    in_maps = []
    rhs_gt = {}
    rhs_pred = {}
    for b in range(B):
        # rhs matrices are shared by the two cores of a batch; build once
        _, rhs_gt[b] = _augment(pred[b][:1], gt[b])
        _, rhs_pred[b] = _augment(gt[b][:1], pred[b])
    for c in range(NCORES):
        b, h = c // 2, c % 2
        rows = slice(h * HALF, (h + 1) * HALF)
        lhsT_E, _ = _augment(pred[b][rows], gt[b][:1])
        lhsT_F, _ = _augment(gt[b][rows], pred[b][:1])
        rE, rF = rhs_gt[b], rhs_pred[b]
        if variant.startswith("pack2d"):
            lhsT_E, rE = _pack2d(lhsT_E, rE)
            lhsT_F, rF = _pack2d(lhsT_F, rF)
        elif variant.startswith("pack2"):
            lhsT_E, rE = _pack2(lhsT_E, rE)
            lhsT_F, rF = _pack2(lhsT_F, rF)
        in_maps.append({
            "lhsT_E": lhsT_E, "rhs_E": rE,
            "lhsT_F": lhsT_F, "rhs_F": rF,
        })
    return in_maps


def _unstage(arr):
    """[128, 32] staging -> [4096] vector with row index t*128+p."""
    return np.asarray(arr, np.float32).T.reshape(-1)


def kernel(pred, gt):
    pred = np.asarray(pred, dtype=np.float32)
    gt = np.asarray(gt, dtype=np.float32)
    assert pred.shape == (B, N, 3) and gt.shape == (B, M, 3)

    if "nc" not in _CACHE:
        _CACHE["nc"] = _build_program(variant=VARIANT)
    nc = _CACHE["nc"]

    in_maps = _make_in_maps(pred, gt, variant=VARIANT)
    res = run_bass_kernel_spmd(nc, in_maps, list(range(NCORES)))

    loss_terms = []
    for b in range(B):
        p2g = np.concatenate([_unstage(res.results[2 * b]["outE"]),
                              _unstage(res.results[2 * b + 1]["outE"])])
        g2p = np.concatenate([_unstage(res.results[2 * b]["outF"]),
                              _unstage(res.results[2 * b + 1]["outF"])])
        loss_terms.append(p2g.mean(dtype=np.float64)
                          + g2p.mean(dtype=np.float64)
                          + np.float64(p2g.max()))
    return np.float32(np.mean(loss_terms))



# revision 2
# speedup vs baseline: 5.5997x; 5.5997x over previous
"""ChamferLoss kernel for 8 Trainium2 NeuronCores.

Problem: pred (4,8192,3) f32, gt (4,8192,3) f32 ->
  loss = mean_b[ mean(pred2gt_b) + mean(gt2pred_b) + max(pred2gt_b) ]   (scalar f32)
where pred2gt[b,i] = min_j ||pred[b,i]-gt[b,j]||^2 and gt2pred[b,j] = min_i (same).

Work split: one (batch, direction) pair per core — core 2b computes pred2gt for
batch b, core 2b+1 computes gt2pred.  The SPMD Bass program is direction-
agnostic ("row-mins of an 8192x8192 distance matrix"); direction is purely data
routing.

Wire-cost design (the axon device link has ~80 ms sync latency and ~100 MB/s
bandwidth, which dwarfs the ~0.3 ms of actual HW compute):
  * Host ships ONLY the raw points, each byte exactly once: core c receives
    its own lhs point set transposed (3,8192) f32 — 786 KB total.
  * A cached on-device prep jit (shard_map) ppermutes the partner core's
    points across NeuronLink and builds the augmented split-precision
    matmul operands (lhsT/rhs, bf16) entirely on device.
  * The Bass program runs via a cached jit of the bass_exec custom call
    (run_bass_kernel_spmd's axon path rebuilds its jit closure every call,
    which re-traces + re-compiles; caching it is most of the win).
  * A post jit reduces the per-core row-mins to the final scalar on device;
    the only D2H is that scalar.  All dispatches are async; the single sync
    point is the scalar fetch.

Distance computation: one K=18 bf16 matmul per tile via the augmented split-
precision form  d = nx + ny - 2 x.y  with x = xh + xl (bf16 hi/lo split) and
norms split into 3 bf16 parts; PSUM accumulates in fp32 (abs err ~1e-4).
The K rows are duplicated into PE row groups 0 and 32 so two matmuls cover
two 512-col strips concurrently.

Reduction: per 1024-column PSUM pair, ScalarE copies one unit to SBUF; a
custom fused DVE op (min body + min accumulate) reduces the other unit
against the copy in a single pass, draining PSUM through both the DVE and
ACT read ports concurrently.
"""

from contextlib import ExitStack

import numpy as np
import ml_dtypes

import jax
import jax.numpy as jnp
from jax.sharding import Mesh, PartitionSpec, NamedSharding
from jax.experimental.shard_map import shard_map

import concourse.tile as tile
from concourse import bacc, mybir, bass2jax
from concourse import dve_ops
from concourse.dve_ops import DveOp
from concourse.dve_spec import Spec, Src0, Src1, C0, minn, lower
from concourse.dve_uop import DveOpSpec

B = 4
N = 8192          # points per batch per tensor
NCORES = 8
K = 18            # augmented contraction rows
KP = 50           # packed rows: K at partitions 0..17 and 32..49
ITILE = 128       # rows per matmul tile
NSTRIP = 512      # matmul moving free dim
NT = N // ITILE   # 64 i-tiles
BIG = 3.0e38

_bf16 = ml_dtypes.bfloat16


# --------------------------------------------------------------------------- #
# Custom fused DVE op: out = min(in0, in1); accum_out = min(s0, min_k out)
# --------------------------------------------------------------------------- #

def _ttmin_ref(in0, in1, s0, s1, imm2):
    out = np.minimum(in0.astype(np.float32), in1.astype(np.float32))
    s0v = s0 if np.ndim(s0) == 0 else np.asarray(s0).reshape(-1)
    return out, np.minimum(out.min(axis=-1), s0v)


def _register_min_op() -> DveOp:
    name = "TT_MIN_RED_ANT"
    for o in dve_ops.OPS:
        if o.name == name:
            return o
    spec = Spec(body=minn(Src0, Src1), accum=minn, accum_init=C0, reference=_ttmin_ref)
    shas = {}
    for ver in ("v3", "v4"):
        try:
            s = DveOpSpec(name=name, opcode=0, uops=lower(spec, ver=ver), rd1_en=True)
            shas[ver] = s.sha(ver)
        except Exception:
            pass
    op = DveOp(name, spec, subdim=False, uops_sha=shas)
    dve_ops.OPS.append(op)
    dve_ops._SUB_OPCODE_FOR_NAME[name] = dve_ops._CUSTOM_DVE_ROW_BASE + len(dve_ops.OPS) - 1
    dve_ops.CUSTOM_DVE_SPECS[name] = spec
    return op


# --------------------------------------------------------------------------- #
# Bass program (identical SPMD program on all 8 cores): row-mins of the
# 8192x8192 squared-distance matrix given packed lhsT/rhs.
# --------------------------------------------------------------------------- #

def _build_program():
    op = _register_min_op()
    nc = bacc.Bacc("TRN2", target_bir_lowering=False, debug=False,
                   num_devices=NCORES)

    lhsT_in = nc.dram_tensor("lhsT", [KP, N], mybir.dt.bfloat16,
                             kind="ExternalInput").ap()
    rhs_in = nc.dram_tensor("rhs", [KP, N], mybir.dt.bfloat16,
                            kind="ExternalInput").ap()
    out = nc.dram_tensor("out", [ITILE, NT], mybir.dt.float32,
                         kind="ExternalOutput").ap()

    with tile.TileContext(nc) as tc:
        with ExitStack() as ctx:
            inp = ctx.enter_context(tc.tile_pool(name="inp", bufs=1))
            psum = ctx.enter_context(tc.tile_pool(name="psum", bufs=2, space="PSUM"))
            acp = ctx.enter_context(tc.tile_pool(name="acp", bufs=3))
            scr = ctx.enter_context(tc.tile_pool(name="scr", bufs=3))
            stp = ctx.enter_context(tc.tile_pool(name="stp", bufs=3))
            ost = ctx.enter_context(tc.tile_pool(name="ost", bufs=1))

            lhsT = inp.tile([KP, N], mybir.dt.bfloat16, tag="lhsT")
            nc.sync.dma_start(out=lhsT[:], in_=lhsT_in[:])
            rhs = inp.tile([KP, N], mybir.dt.bfloat16, tag="rhs")
            nc.sync.dma_start(out=rhs[:], in_=rhs_in[:])

            outstage = ost.tile([ITILE, NT], mybir.dt.float32, tag="outstage")
            for t in range(NT):
                strip = stp.tile([ITILE, 4], mybir.dt.float32, tag="strip")
                cp = None
                for u in range(8):  # 1024-col units (2 strips, one per group)
                    pt = psum.tile([ITILE, 1024], mybir.dt.float32,
                                   tag="pt", bufs=4)
                    for g in range(2):
                        j0 = (2 * u + g) * NSTRIP
                        nc.tensor.matmul(
                            pt[:, g * NSTRIP:(g + 1) * NSTRIP],
                            lhsT[32 * g:32 * g + K, t * ITILE:(t + 1) * ITILE],
                            rhs[32 * g:32 * g + K, j0:j0 + NSTRIP],
                            start=True, stop=True)
                    if u % 2 == 0:
                        cp = acp.tile([ITILE, 1024], mybir.dt.float32, tag="cp")
                        nc.scalar.copy(cp[:], pt[:])
                    else:
                        sc = scr.tile([ITILE, 1024], mybir.dt.bfloat16, tag="sc")
                        nc.vector._custom_dve(
                            op, out=sc[:], in0=pt[:], in1=cp[:],
                            s0=BIG,
                            accum_out=strip[:, u // 2:u // 2 + 1])
                nc.vector.tensor_reduce(
                    outstage[:, t:t + 1], strip[:],
                    axis=mybir.AxisListType.X, op=mybir.AluOpType.min)
            nc.sync.dma_start(out=out[:], in_=outstage[:])

    nc.compile()
    return nc


# --------------------------------------------------------------------------- #
# Cached device pipeline: prep jit -> bass jit -> post jit
# --------------------------------------------------------------------------- #

_CACHE: dict = {}


def _split3(v):
    a = v.astype(jnp.bfloat16)
    r = v - a.astype(jnp.float32)
    b = r.astype(jnp.bfloat16)
    r2 = r - b.astype(jnp.float32)
    return a, b, r2.astype(jnp.bfloat16)


def _prep_core(x):
    """x: (3, N) f32 — this core's lhs points. Builds packed (KP, N) bf16
    lhsT and rhs; rhs points come from the paired core via ppermute."""
    y = jax.lax.ppermute(x, "core", [(i, i ^ 1) for i in range(NCORES)])
    xh = x.astype(jnp.bfloat16)
    xl = (x - xh.astype(jnp.float32)).astype(jnp.bfloat16)
    yh = y.astype(jnp.bfloat16)
    yl = (y - yh.astype(jnp.float32)).astype(jnp.bfloat16)
    xe = xh.astype(jnp.float32) + xl.astype(jnp.float32)
    ye = yh.astype(jnp.float32) + yl.astype(jnp.float32)
    nx = jnp.sum(xe * xe, axis=0)
    ny = jnp.sum(ye * ye, axis=0)
    nxa, nxb, nxc = _split3(nx)
    nya, nyb, nyc = _split3(ny)
    one = jnp.ones((1, N), jnp.bfloat16)
    y2h = (-2.0 * yh.astype(jnp.float32)).astype(jnp.bfloat16)
    y2l = (-2.0 * yl.astype(jnp.float32)).astype(jnp.bfloat16)
    lblk = jnp.concatenate(
        [xh, xh, xl, xl, nxa[None], nxb[None], nxc[None], one, one, one], axis=0)
    rblk = jnp.concatenate(
        [y2h, y2l, y2h, y2l, one, one, one, nya[None], nyb[None], nyc[None]],
        axis=0)
    z = jnp.zeros((32 - K, N), jnp.bfloat16)
    return (jnp.concatenate([lblk, z, lblk], axis=0),
            jnp.concatenate([rblk, z, rblk], axis=0))


def _post(o):
    """o: (NCORES*ITILE, NT) f32 sharded on axis 0 — staged row-mins.
    Mean/max are order-agnostic, so no unstaging needed."""
    v = o.reshape(NCORES, ITILE, NT)
    m = jnp.mean(v, axis=(1, 2))
    mx = jnp.max(v, axis=(1, 2))
    return jnp.mean(m[0::2] + m[1::2] + mx[0::2])


def _build_pipeline():
    nc = _build_program()
    bass2jax.install_neuronx_cc_hook()

    partition_name = (nc.partition_id_tensor.name
                      if nc.partition_id_tensor else None)
    in_names, out_names, out_avals = [], [], []
    for alloc in nc.m.functions[0].allocations:
        if not isinstance(alloc, mybir.MemoryLocationSet):
            continue
        name = alloc.memorylocations[0].name
        if alloc.kind == "ExternalInput":
            if name != partition_name:
                in_names.append(name)
        elif alloc.kind == "ExternalOutput":
            out_names.append(name)
            out_avals.append(jax.core.ShapedArray(
                tuple(alloc.tensor_shape), mybir.dt.np(alloc.dtype)))
    assert in_names == ["lhsT", "rhs"] and out_names == ["out"], \
        (in_names, out_names)
    n_params = len(in_names)
    n_outs = len(out_names)
    in_names_full = in_names + out_names + (
        [partition_name] if partition_name else [])

    def _body(*args):
        operands = list(args)
        if partition_name is not None:
            operands.append(bass2jax.partition_id_tensor())
        return tuple(bass2jax._bass_exec_p.bind(
            *operands, out_avals=tuple(out_avals),
            in_names=tuple(in_names_full), out_names=tuple(out_names),
            lowering_input_output_aliases=(),
            sim_require_finite=True, sim_require_nnan=True, nc=nc))

    devices = jax.devices()[:NCORES]
    mesh = Mesh(np.asarray(devices), ("core",))
    shard = NamedSharding(mesh, PartitionSpec("core"))
    donate = tuple(range(n_params, n_params + n_outs))
    bass_jit = jax.jit(
        shard_map(_body, mesh=mesh,
                  in_specs=(PartitionSpec("core"),) * (n_params + n_outs),
                  out_specs=(PartitionSpec("core"),) * n_outs,
                  check_rep=False),
        donate_argnums=donate, keep_unused=True)

    prep_jit = jax.jit(
        shard_map(_prep_core, mesh=mesh, in_specs=(PartitionSpec("core"),),
                  out_specs=(PartitionSpec("core"), PartitionSpec("core")),
                  check_rep=False))
    zeros_jit = jax.jit(
        lambda: jnp.zeros((NCORES * ITILE, NT), jnp.float32),
        out_shardings=shard)
    post_jit = jax.jit(_post)

    return {"bass_jit": bass_jit, "prep_jit": prep_jit,
            "zeros_jit": zeros_jit, "post_jit": post_jit, "shard": shard}


def kernel(pred, gt):
    pred = np.ascontiguousarray(np.asarray(pred, dtype=np.float32))
    gt = np.ascontiguousarray(np.asarray(gt, dtype=np.float32))
    assert pred.shape == (B, N, 3) and gt.shape == (B, N, 3)

    if "pipe" not in _CACHE:
        _CACHE["pipe"] = _build_pipeline()
    p = _CACHE["pipe"]

    # Core 2b gets pred[b] (computes pred2gt); core 2b+1 gets gt[b].
    xT = np.empty((NCORES * 3, N), np.float32)
    for c in range(NCORES):
        b, o = divmod(c, 2)
        src = pred[b] if o == 0 else gt[b]
        xT[3 * c:3 * c + 3] = src.T

    d = jax.device_put(xT, p["shard"])
    lhsT, rhs = p["prep_jit"](d)
    (out,) = p["bass_jit"](lhsT, rhs, p["zeros_jit"]())
    return np.float32(p["post_jit"](out))


# revision 5
# speedup vs baseline: 10.6393x; 1.9000x over previous
"""ChamferLoss kernel for 8 Trainium2 NeuronCores.

Problem: pred (4,8192,3) f32, gt (4,8192,3) f32 ->
  loss = mean_b[ mean(pred2gt_b) + mean(gt2pred_b) + max(pred2gt_b) ]   (scalar f32)
where pred2gt[b,i] = min_j ||pred[b,i]-gt[b,j]||^2 and gt2pred[b,j] = min_i (same).

Work split: one (batch, direction) pair per core — core 2b computes pred2gt for
batch b, core 2b+1 computes gt2pred.  The SPMD Bass program is direction-
agnostic ("row-mins of an 8192x8192 distance matrix"); direction is purely data
routing.

Wire-cost design (the axon device link has ~80 ms sync latency and ~100 MB/s
bandwidth, which dwarfs the ~0.3 ms of actual HW compute):
  * Host ships ONLY the raw points, each byte exactly once: core c receives
    its own lhs point set transposed (3,8192) f32 — 786 KB total.
  * A cached on-device prep jit (shard_map) ppermutes the partner core's
    points across NeuronLink and builds the augmented split-precision
    matmul operands (lhsT/rhs, bf16) entirely on device.
  * The Bass program runs via a cached jit of the bass_exec custom call
    (run_bass_kernel_spmd's axon path rebuilds its jit closure every call,
    which re-traces + re-compiles; caching it is most of the win).
  * A post jit reduces the per-core row-mins to the final scalar on device;
    the only D2H is that scalar.  All dispatches are async; the single sync
    point is the scalar fetch.

Distance computation: one K=18 bf16 matmul per tile via the augmented split-
precision form  d = nx + ny - 2 x.y  with x = xh + xl (bf16 hi/lo split) and
norms split into 3 bf16 parts; PSUM accumulates in fp32 (abs err ~1e-4).
The K rows are duplicated into PE row groups 0 and 32 so two matmuls cover
two 512-col strips concurrently.

Reduction: per 1024-column PSUM pair, ScalarE copies one unit to SBUF; a
custom fused DVE op (min body + min accumulate) reduces the other unit
against the copy in a single pass, draining PSUM through both the DVE and
ACT read ports concurrently.
"""

from contextlib import ExitStack

import numpy as np
import ml_dtypes

import jax
import jax.numpy as jnp
from jax.sharding import Mesh, PartitionSpec, NamedSharding
from jax.experimental.shard_map import shard_map

import concourse.tile as tile
from concourse import bacc, mybir, bass2jax
from concourse import dve_ops
from concourse.dve_ops import DveOp
from concourse.dve_spec import Spec, Src0, Src1, C0, minn, lower
from concourse.dve_uop import DveOpSpec

B = 4
N = 8192          # points per batch per tensor
NCORES = 8
K = 18            # augmented contraction rows
KP = 50           # packed rows: K at partitions 0..17 and 32..49
ITILE = 128       # rows per matmul tile
NSTRIP = 512      # matmul moving free dim
NT = N // ITILE   # 64 i-tiles
BIG = 3.0e38

_bf16 = ml_dtypes.bfloat16


# --------------------------------------------------------------------------- #
# Custom fused DVE op: out = min(in0, in1); accum_out = min(s0, min_k out)
# --------------------------------------------------------------------------- #

def _ttmin_ref(in0, in1, s0, s1, imm2):
    out = np.minimum(in0.astype(np.float32), in1.astype(np.float32))
    s0v = s0 if np.ndim(s0) == 0 else np.asarray(s0).reshape(-1)
    return out, np.minimum(out.min(axis=-1), s0v)


def _register_min_op() -> DveOp:
    name = "TT_MIN_RED_ANT"
    for o in dve_ops.OPS:
        if o.name == name:
            return o
    spec = Spec(body=minn(Src0, Src1), accum=minn, accum_init=C0, reference=_ttmin_ref)
    shas = {}
    for ver in ("v3", "v4"):
        try:
            s = DveOpSpec(name=name, opcode=0, uops=lower(spec, ver=ver), rd1_en=True)
            shas[ver] = s.sha(ver)
        except Exception:
            pass
    op = DveOp(name, spec, subdim=False, uops_sha=shas)
    dve_ops.OPS.append(op)
    dve_ops._SUB_OPCODE_FOR_NAME[name] = dve_ops._CUSTOM_DVE_ROW_BASE + len(dve_ops.OPS) - 1
    dve_ops.CUSTOM_DVE_SPECS[name] = spec
    return op


# --------------------------------------------------------------------------- #
# Bass program (identical SPMD program on all 8 cores): row-mins of the
# 8192x8192 squared-distance matrix given packed lhsT/rhs.
# --------------------------------------------------------------------------- #

def _build_program():
    op = _register_min_op()
    nc = bacc.Bacc("TRN2", target_bir_lowering=False, debug=False,
                   num_devices=NCORES)

    lhsT_in = nc.dram_tensor("lhsT", [KP, N], mybir.dt.bfloat16,
                             kind="ExternalInput").ap()
    rhs_in = nc.dram_tensor("rhs", [KP, N], mybir.dt.bfloat16,
                            kind="ExternalInput").ap()
    out = nc.dram_tensor("out", [ITILE, NT], mybir.dt.float32,
                         kind="ExternalOutput").ap()

    with tile.TileContext(nc) as tc:
        with ExitStack() as ctx:
            inp = ctx.enter_context(tc.tile_pool(name="inp", bufs=1))
            psum = ctx.enter_context(tc.tile_pool(name="psum", bufs=2, space="PSUM"))
            acp = ctx.enter_context(tc.tile_pool(name="acp", bufs=3))
            scr = ctx.enter_context(tc.tile_pool(name="scr", bufs=3))
            stp = ctx.enter_context(tc.tile_pool(name="stp", bufs=3))
            ost = ctx.enter_context(tc.tile_pool(name="ost", bufs=1))

            lhsT = inp.tile([KP, N], mybir.dt.bfloat16, tag="lhsT")
            nc.sync.dma_start(out=lhsT[:], in_=lhsT_in[:])
            rhs = inp.tile([KP, N], mybir.dt.bfloat16, tag="rhs")
            nc.sync.dma_start(out=rhs[:], in_=rhs_in[:])

            outstage = ost.tile([ITILE, NT], mybir.dt.float32, tag="outstage")
            for t in range(NT):
                strip = stp.tile([ITILE, 4], mybir.dt.float32, tag="strip")
                cp = None
                for u in range(8):  # 1024-col units (2 strips, one per group)
                    pt = psum.tile([ITILE, 1024], mybir.dt.float32,
                                   tag="pt", bufs=4)
                    for g in range(2):
                        j0 = (2 * u + g) * NSTRIP
                        nc.tensor.matmul(
                            pt[:, g * NSTRIP:(g + 1) * NSTRIP],
                            lhsT[32 * g:32 * g + K, t * ITILE:(t + 1) * ITILE],
                            rhs[32 * g:32 * g + K, j0:j0 + NSTRIP],
                            start=True, stop=True)
                    if u % 2 == 0:
                        cp = acp.tile([ITILE, 1024], mybir.dt.float32, tag="cp")
                        nc.scalar.copy(cp[:], pt[:])
                    else:
                        sc = scr.tile([ITILE, 1024], mybir.dt.bfloat16, tag="sc")
                        nc.vector._custom_dve(
                            op, out=sc[:], in0=pt[:], in1=cp[:],
                            s0=BIG,
                            accum_out=strip[:, u // 2:u // 2 + 1])
                nc.vector.tensor_reduce(
                    outstage[:, t:t + 1], strip[:],
                    axis=mybir.AxisListType.X, op=mybir.AluOpType.min)
            nc.sync.dma_start(out=out[:], in_=outstage[:])

    nc.compile()
    return nc


# --------------------------------------------------------------------------- #
# Cached device pipeline: prep jit -> bass jit -> post jit
# --------------------------------------------------------------------------- #

_CACHE: dict = {}


_VELT_C = np.float32(65537.0)  # 2^16 + 1


def _velt(v):
    """Round f32 -> bf16-representable value, keeping f32 dtype, via Veltkamp
    splitting (pure f32 mul/sub).  jnp converts can't be used for values
    feeding further f32 math: the neuron compiler folds
    bf16(x - f32(bf16(x))) convert chains into bf16 arithmetic, zeroing the
    residual; and integer bitcast tricks ICE the walrus backend."""
    p = v * _VELT_C
    q = v - p
    return p + q


def _split3(v):
    """f32 vector -> 3 bf16 rows summing to ~v (a,b as exact-value converts,
    c as a final genuine rounding)."""
    a = _velt(v)
    r = v - a
    b = _velt(r)
    r2 = r - b
    return a.astype(jnp.bfloat16), b.astype(jnp.bfloat16), r2.astype(jnp.bfloat16)


def _prep_core(x):
    """x: (3, N) f32 — this core's lhs points. Builds packed (KP, N) bf16
    lhsT and rhs; rhs points come from the paired core via ppermute."""
    y = jax.lax.ppermute(x, "core", [(i, i ^ 1) for i in range(NCORES)])
    xh = _velt(x)
    xl32 = x - xh
    xl = xl32.astype(jnp.bfloat16)
    yh = _velt(y)
    yl32 = y - yh
    yl = yl32.astype(jnp.bfloat16)
    xe = xh + _velt(xl32)
    ye = yh + _velt(yl32)
    nx = jnp.sum(xe * xe, axis=0)
    ny = jnp.sum(ye * ye, axis=0)
    nxa, nxb, nxc = _split3(nx)
    nya, nyb, nyc = _split3(ny)
    one = jnp.ones((1, N), jnp.bfloat16)
    y2h = (-2.0 * yh).astype(jnp.bfloat16)
    y2l = (-2.0 * _velt(yl32)).astype(jnp.bfloat16)
    lblk = jnp.concatenate(
        [xh.astype(jnp.bfloat16), xh.astype(jnp.bfloat16), xl, xl,
         nxa[None], nxb[None], nxc[None], one, one, one], axis=0)
    rblk = jnp.concatenate(
        [y2h, y2l, y2h, y2l, one, one, one, nya[None], nyb[None], nyc[None]],
        axis=0)
    z = jnp.zeros((32 - K, N), jnp.bfloat16)
    return (jnp.concatenate([lblk, z, lblk], axis=0),
            jnp.concatenate([rblk, z, rblk], axis=0))


def _post(o):
    """o: (NCORES*ITILE, NT) f32 sharded on axis 0 — staged row-mins.
    Mean/max are order-agnostic, so no unstaging needed."""
    v = o.reshape(NCORES, ITILE, NT)
    m = jnp.mean(v, axis=(1, 2))
    mx = jnp.max(v, axis=(1, 2))
    return jnp.mean(m[0::2] + m[1::2] + mx[0::2])


def _build_pipeline():
    nc = _build_program()
    bass2jax.install_neuronx_cc_hook()

    partition_name = (nc.partition_id_tensor.name
                      if nc.partition_id_tensor else None)
    in_names, out_names, out_avals = [], [], []
    for alloc in nc.m.functions[0].allocations:
        if not isinstance(alloc, mybir.MemoryLocationSet):
            continue
        name = alloc.memorylocations[0].name
        if alloc.kind == "ExternalInput":
            if name != partition_name:
                in_names.append(name)
        elif alloc.kind == "ExternalOutput":
            out_names.append(name)
            out_avals.append(jax.core.ShapedArray(
                tuple(alloc.tensor_shape), mybir.dt.np(alloc.dtype)))
    assert in_names == ["lhsT", "rhs"] and out_names == ["out"], \
        (in_names, out_names)
    n_params = len(in_names)
    n_outs = len(out_names)
    in_names_full = in_names + out_names + (
        [partition_name] if partition_name else [])

    def _body(*args):
        operands = list(args)
        if partition_name is not None:
            operands.append(bass2jax.partition_id_tensor())
        return tuple(bass2jax._bass_exec_p.bind(
            *operands, out_avals=tuple(out_avals),
            in_names=tuple(in_names_full), out_names=tuple(out_names),
            lowering_input_output_aliases=(),
            sim_require_finite=True, sim_require_nnan=True, nc=nc))

    devices = jax.devices()[:NCORES]
    mesh = Mesh(np.asarray(devices), ("core",))
    shard = NamedSharding(mesh, PartitionSpec("core"))
    donate = tuple(range(n_params, n_params + n_outs))
    bass_jit = jax.jit(
        shard_map(_body, mesh=mesh,
                  in_specs=(PartitionSpec("core"),) * (n_params + n_outs),
                  out_specs=(PartitionSpec("core"),) * n_outs,
                  check_rep=False),
        donate_argnums=donate, keep_unused=True)

    prep_jit = jax.jit(
        shard_map(_prep_core, mesh=mesh, in_specs=(PartitionSpec("core"),),
                  out_specs=(PartitionSpec("core"), PartitionSpec("core")),
                  check_rep=False))
    zeros_jit = jax.jit(
        lambda: jnp.zeros((NCORES * ITILE, NT), jnp.float32),
        out_shardings=shard)
    post_jit = jax.jit(_post)

    return {"bass_jit": bass_jit, "prep_jit": prep_jit,
            "zeros_jit": zeros_jit, "post_jit": post_jit, "shard": shard}


def kernel(pred, gt):
    pred = np.ascontiguousarray(np.asarray(pred, dtype=np.float32))
    gt = np.ascontiguousarray(np.asarray(gt, dtype=np.float32))
    assert pred.shape == (B, N, 3) and gt.shape == (B, N, 3)

    if "pipe" not in _CACHE:
        _CACHE["pipe"] = _build_pipeline()
    p = _CACHE["pipe"]

    # Core 2b gets pred[b] (computes pred2gt); core 2b+1 gets gt[b].
    xT = np.empty((NCORES * 3, N), np.float32)
    for c in range(NCORES):
        b, o = divmod(c, 2)
        src = pred[b] if o == 0 else gt[b]
        xT[3 * c:3 * c + 3] = src.T

    d = jax.device_put(xT, p["shard"])
    lhsT, rhs = p["prep_jit"](d)
    (out,) = p["bass_jit"](lhsT, rhs, p["zeros_jit"]())
    return np.float32(p["post_jit"](out))
